# revision 5
# baseline (speedup 1.0000x reference)
"""Single-head attention (B=8, N=2048, E=1024) on 8 TRN2 NeuronCores.

Sharding: data-parallel over batch — core i computes batch element i fully.
Host-side prep transposes x and W so the device kernel needs no transposes:
every matmul operand arrives with its contraction dim on SBUF partitions.

Per-core dataflow (all matmul compute in bf16, f32 PSUM accumulation):
  qT[f,n] = WT_lhsT.T @ xT_rhs      (projection, f on partitions)
  kT[f,n] = same
  v[n,e]  = xT_lhsT.T @ WTv_rhs     (natural layout, n on partitions)
  scoresT[j,i] = kT_lhsT.T @ qT_rhs ; expT = exp(scale*scoresT)  (ScalarE)
  denom[i] = ones-matmul over j-partitions of DVE-reduced exp sums
  out[i,e] = (expT_lhsT.T @ v_rhs) * (1/denom)
Softmax skips max-subtraction: scores are ~N(0,1) (max |s| < ~8), exp is
safe in f32 and softmax is shift-invariant.
"""

import numpy as np
import ml_dtypes

P = 128
E = 1024
N = 2048
F = 3072
KO = E // P      # 8 contraction subtiles
NT = N // P      # 16 row tiles
NCH = N // 512   # 4 chunks of 512
SCALE = 0.03125  # 1/sqrt(1024)

_CACHE = {}


def _build():
    import concourse.bacc as bacc
    import concourse.tile as tile
    import concourse.mybir as mybir

    f32 = mybir.dt.float32
    bf16 = mybir.dt.bfloat16
    AF = mybir.ActivationFunctionType
    Alu = mybir.AluOpType

    nc = bacc.Bacc("TRN2", target_bir_lowering=False, debug=False, num_devices=8)
    xT_d = nc.dram_tensor("xT", [E, N], bf16, kind="ExternalInput")
    WT_d = nc.dram_tensor("WT", [E, F], bf16, kind="ExternalInput")
    bqk_d = nc.dram_tensor("b_qk", [P, 16], f32, kind="ExternalInput")
    bv_d = nc.dram_tensor("b_v", [P, E], f32, kind="ExternalInput")
    out_d = nc.dram_tensor("out", [N, E], f32, kind="ExternalOutput")

    xT_r = xT_d.ap().rearrange("(ko p) n -> ko p n", p=P)
    WT_r = WT_d.ap().rearrange("(ko p) f -> ko p f", p=P)
    out_r = out_d.ap().rearrange("(it p) e -> it p e", p=P)

    with tile.TileContext(nc) as tc:
        with (
            tc.tile_pool(name="const", bufs=1) as const,
            tc.tile_pool(name="qkv", bufs=1) as qkv,
        ):
            bqk_t = const.tile([P, 16], f32, tag="bqk")
            nc.sync.dma_start(bqk_t[:], bqk_d.ap())
            bv_t = const.tile([P, E], f32, tag="bv")
            nc.sync.dma_start(bv_t[:], bv_d.ap())
            ones_t = const.tile([P, 1], f32, tag="ones")
            nc.vector.memset(ones_t[:], 1.0)

            qT = qkv.tile([P, KO, N], bf16, tag="qT")
            kT = qkv.tile([P, KO, N], bf16, tag="kT")
            vt = qkv.tile([P, NT, E], bf16, tag="v")

            with (
                tc.tile_pool(name="pin", bufs=1) as pin,
                tc.tile_pool(name="pproj", bufs=4, space="PSUM") as pproj,
            ):
                xk, wk = [], []
                for k in range(KO):
                    t = pin.tile([P, N], bf16, tag=f"x{k}")
                    nc.sync.dma_start(t[:], xT_r[k])
                    xk.append(t)
                    t = pin.tile([P, F], bf16, tag=f"w{k}")
                    nc.sync.dma_start(t[:], WT_r[k])
                    wk.append(t)

                # q/k projection -> qT/kT [f(part), n]
                for ch in range(NCH):
                    nsl = slice(ch * 512, (ch + 1) * 512)
                    for ft in range(16):  # 0-7: q rows of W, 8-15: k rows
                        ps = pproj.tile([P, 512], f32, tag="ps")
                        for k in range(KO):
                            nc.tensor.matmul(
                                ps[:],
                                lhsT=wk[k][:, ft * P : (ft + 1) * P],
                                rhs=xk[k][:, nsl],
                                start=(k == 0),
                                stop=(k == KO - 1),
                            )
                        dst = (qT if ft < 8 else kT)[:, ft % 8, nsl]
                        nc.scalar.activation(
                            dst, ps[:], AF.Identity, bias=bqk_t[:, ft : ft + 1], scale=1.0
                        )

                # v projection -> v [n(part), e]
                for nt in range(NT):
                    for ch2 in range(2):
                        esl = slice(ch2 * 512, (ch2 + 1) * 512)
                        ps = pproj.tile([P, 512], f32, tag="ps")
                        for k in range(KO):
                            nc.tensor.matmul(
                                ps[:],
                                lhsT=xk[k][:, nt * P : (nt + 1) * P],
                                rhs=wk[k][:, 2 * E + ch2 * 512 : 2 * E + (ch2 + 1) * 512],
                                start=(k == 0),
                                stop=(k == KO - 1),
                            )
                        nc.vector.tensor_tensor(
                            out=vt[:, nt, esl],
                            in0=ps[:],
                            in1=bv_t[:, esl],
                            op=Alu.add,
                        )

            with (
                tc.tile_pool(name="attn", bufs=2) as attn,
                tc.tile_pool(name="psc", bufs=2, space="PSUM") as psc,
                tc.tile_pool(name="pnum", bufs=4, space="PSUM") as pnum,
                tc.tile_pool(name="pden", bufs=2, space="PSUM") as pden,
            ):
                for ic in range(NCH):
                    isl = slice(ic * 512, (ic + 1) * 512)
                    expT = attn.tile([P, NT, 512], bf16, tag="expT")
                    for jt in range(NT):
                        ps = psc.tile([P, 512], f32, tag="ps_s")
                        for k in range(KO):
                            nc.tensor.matmul(
                                ps[:],
                                lhsT=kT[:, k, jt * P : (jt + 1) * P],
                                rhs=qT[:, k, isl],
                                start=(k == 0),
                                stop=(k == KO - 1),
                            )
                        nc.scalar.activation(expT[:, jt, :], ps[:], AF.Exp, scale=SCALE)

                    # partial softmax denominators: sum over j-tiles (DVE), then
                    # over the remaining 128 j-partitions (ones-matmul)
                    sume = attn.tile([P, 512], f32, tag="sume")
                    nc.vector.reduce_sum(
                        sume[:],
                        expT.rearrange("p j i -> p i j"),
                        axis=mybir.AxisListType.X,
                    )
                    for isub in range(4):
                        it = ic * 4 + isub
                        psd = pden.tile([P, 1], f32, tag="ps_d")
                        nc.tensor.matmul(
                            psd[:],
                            lhsT=sume[:, isub * P : (isub + 1) * P],
                            rhs=ones_t[:],
                            start=True,
                            stop=True,
                        )
                        rden = attn.tile([P, 1], f32, tag="rden", bufs=4)
                        nc.vector.reciprocal(rden[:], psd[:])
                        osb = attn.tile([P, E], f32, tag="osb", bufs=3)
                        for ch2 in range(2):
                            esl = slice(ch2 * 512, (ch2 + 1) * 512)
                            ps = pnum.tile([P, 512], f32, tag="ps_n")
                            for jt in range(NT):
                                nc.tensor.matmul(
                                    ps[:],
                                    lhsT=expT[:, jt, isub * P : (isub + 1) * P],
                                    rhs=vt[:, jt, esl],
                                    start=(jt == 0),
                                    stop=(jt == NT - 1),
                                )
                            nc.vector.tensor_scalar_mul(osb[:, esl], ps[:], rden[:])
                        nc.sync.dma_start(out_r[it], osb[:])
    nc.compile()
    return nc


def get_nc():
    if "nc" not in _CACHE:
        _CACHE["nc"] = _build()
    return _CACHE["nc"]


def prepare_in_maps(x, W_qkv, b_qkv):
    bf = ml_dtypes.bfloat16
    x = np.asarray(x, dtype=np.float32)
    W = np.asarray(W_qkv, dtype=np.float32)
    b = np.asarray(b_qkv, dtype=np.float32)
    assert x.shape == (8, N, E) and W.shape == (F, E) and b.shape == (F,)
    xT = np.ascontiguousarray(np.transpose(x, (0, 2, 1))).astype(bf)  # [8, E, N]
    WT = np.ascontiguousarray(W.T).astype(bf)  # [E, F]
    bqk = np.ascontiguousarray(b[: 2 * E].reshape(16, P).T)  # [P, 16]
    bv = np.ascontiguousarray(np.broadcast_to(b[2 * E :], (P, E)))  # [P, E]
    return [{"xT": xT[i], "WT": WT, "b_qk": bqk, "b_v": bv} for i in range(8)]


def kernel(x, W_qkv, b_qkv):
    from concourse.bass_utils import run_bass_kernel_spmd

    nc = get_nc()
    in_maps = prepare_in_maps(x, W_qkv, b_qkv)
    res = run_bass_kernel_spmd(nc, in_maps, core_ids=list(range(8)))
    return np.stack([res.results[i]["out"] for i in range(8)], axis=0)


# revision 7
# speedup vs baseline: 1.1079x; 1.1079x over previous
"""Single-head attention (B=8, N=2048, E=1024) on 8 TRN2 NeuronCores.

Sharding: data-parallel over batch — core i computes batch element i fully.
Host-side prep transposes x and W so the device kernel needs no transposes:
every matmul operand arrives with its contraction dim on SBUF partitions.

Per-core dataflow (all matmul compute in bf16, f32 PSUM accumulation):
  qT[f,n] = WT_lhsT.T @ xT_rhs      (projection, f on partitions)
  kT[f,n] = same
  v[n,e]  = xT_lhsT.T @ WTv_rhs     (natural layout, n on partitions)
  scoresT[j,i] = kT_lhsT.T @ qT_rhs ; expT = exp(scale*scoresT)  (ScalarE)
  denom[i] = ones-matmul over j-partitions of DVE-reduced exp sums
  out[i,e] = (expT_lhsT.T @ v_rhs) * (1/denom)
Softmax skips max-subtraction: scores are ~N(0,1) (max |s| < ~8), exp is
safe in f32 and softmax is shift-invariant.
"""

import numpy as np
import ml_dtypes

P = 128
E = 1024
N = 2048
F = 3072
KO = E // P      # 8 contraction subtiles
NT = N // P      # 16 row tiles
NCH = N // 512   # 4 chunks of 512
SCALE = 0.03125  # 1/sqrt(1024)

_CACHE = {}


def _build():
    import concourse.bacc as bacc
    import concourse.tile as tile
    import concourse.mybir as mybir

    f32 = mybir.dt.float32
    bf16 = mybir.dt.bfloat16
    AF = mybir.ActivationFunctionType
    Alu = mybir.AluOpType

    nc = bacc.Bacc("TRN2", target_bir_lowering=False, debug=False, num_devices=8)
    xT_d = nc.dram_tensor("xT", [E, N], bf16, kind="ExternalInput")
    WT_d = nc.dram_tensor("WT", [E, F], bf16, kind="ExternalInput")
    bqk_d = nc.dram_tensor("b_qk", [P, 16], f32, kind="ExternalInput")
    bv_d = nc.dram_tensor("b_v", [P, E], f32, kind="ExternalInput")
    out_d = nc.dram_tensor("out", [N, E], f32, kind="ExternalOutput")

    xT_r = xT_d.ap().rearrange("(ko p) n -> ko p n", p=P)
    WT_r = WT_d.ap().rearrange("(ko p) f -> ko p f", p=P)
    out_r = out_d.ap().rearrange("(it p) e -> it p e", p=P)

    with tile.TileContext(nc) as tc:
        with (
            tc.tile_pool(name="const", bufs=1) as const,
            tc.tile_pool(name="qkv", bufs=1) as qkv,
        ):
            bqk_t = const.tile([P, 16], f32, tag="bqk")
            nc.sync.dma_start(bqk_t[:], bqk_d.ap())
            bv_t = const.tile([P, E], f32, tag="bv")
            nc.sync.dma_start(bv_t[:], bv_d.ap())
            ones_t = const.tile([P, 1], f32, tag="ones")
            nc.vector.memset(ones_t[:], 1.0)

            qT = qkv.tile([P, KO, N], bf16, tag="qT")
            kT = qkv.tile([P, KO, N], bf16, tag="kT")
            vt = qkv.tile([P, NT, E], bf16, tag="v")

            with (
                tc.tile_pool(name="pin", bufs=1) as pin,
                tc.tile_pool(name="pproj", bufs=4, space="PSUM") as pproj,
            ):
                # Chunked input DMAs, issued in the order the projection
                # consumes them, so PE starts ~6us in instead of waiting for
                # the full 10MB load. xc[k][c]: x columns c*512..; wc[k][s]:
                # W columns s*512.. (s 0-1: q, 2-3: k, 4-5: v).
                xc = [[None] * NCH for _ in range(KO)]
                wc = [[None] * 6 for _ in range(KO)]

                def load_x(c):
                    for k in range(KO):
                        t = pin.tile([P, 512], bf16, tag=f"x{k}_{c}")
                        nc.sync.dma_start(t[:], xT_r[k][:, c * 512 : (c + 1) * 512])
                        xc[k][c] = t

                def load_w(s):
                    for k in range(KO):
                        t = pin.tile([P, 512], bf16, tag=f"w{k}_{s}")
                        nc.sync.dma_start(t[:], WT_r[k][:, s * 512 : (s + 1) * 512])
                        wc[k][s] = t

                load_x(0)
                for s in range(4):
                    load_w(s)
                load_x(1)
                load_w(4)
                load_w(5)
                load_x(2)
                load_x(3)

                # q/k projection -> qT/kT [f(part), n]
                for ch in range(NCH):
                    nsl = slice(ch * 512, (ch + 1) * 512)
                    for ft in range(16):  # 0-7: q rows of W, 8-15: k rows
                        ps = pproj.tile([P, 512], f32, tag="ps")
                        for k in range(KO):
                            nc.tensor.matmul(
                                ps[:],
                                lhsT=wc[k][ft // 4][:, (ft % 4) * P : (ft % 4 + 1) * P],
                                rhs=xc[k][ch][:],
                                start=(k == 0),
                                stop=(k == KO - 1),
                            )
                        dst = (qT if ft < 8 else kT)[:, ft % 8, nsl]
                        nc.scalar.activation(
                            dst, ps[:], AF.Identity, bias=bqk_t[:, ft : ft + 1], scale=1.0
                        )

                # v projection -> v [n(part), e]
                for nt in range(NT):
                    for ch2 in range(2):
                        esl = slice(ch2 * 512, (ch2 + 1) * 512)
                        ps = pproj.tile([P, 512], f32, tag="ps")
                        for k in range(KO):
                            nc.tensor.matmul(
                                ps[:],
                                lhsT=xc[k][nt // 4][:, (nt % 4) * P : (nt % 4 + 1) * P],
                                rhs=wc[k][4 + ch2][:],
                                start=(k == 0),
                                stop=(k == KO - 1),
                            )
                        nc.vector.tensor_tensor(
                            out=vt[:, nt, esl],
                            in0=ps[:],
                            in1=bv_t[:, esl],
                            op=Alu.add,
                        )

            with (
                tc.tile_pool(name="attn", bufs=2) as attn,
                tc.tile_pool(name="psc", bufs=2, space="PSUM") as psc,
                tc.tile_pool(name="pnum", bufs=4, space="PSUM") as pnum,
                tc.tile_pool(name="pden", bufs=2, space="PSUM") as pden,
            ):
                # Software pipeline: scores(ic) is emitted before the
                # denominator + numerator of (ic-1), so the DVE exp-sum
                # reduce of chunk ic-1 overlaps with scores matmuls of ic
                # instead of stalling PE.
                def emit_scores(ic):
                    isl = slice(ic * 512, (ic + 1) * 512)
                    expT = attn.tile([P, NT, 512], bf16, tag="expT")
                    for jt in range(NT):
                        ps = psc.tile([P, 512], f32, tag="ps_s")
                        for k in range(KO):
                            nc.tensor.matmul(
                                ps[:],
                                lhsT=kT[:, k, jt * P : (jt + 1) * P],
                                rhs=qT[:, k, isl],
                                start=(k == 0),
                                stop=(k == KO - 1),
                            )
                        nc.scalar.activation(expT[:, jt, :], ps[:], AF.Exp, scale=SCALE)
                    # softmax denominators, step 1: sum over the 16 j-tiles
                    # (free-dim strided reduce on DVE)
                    sume = attn.tile([P, 512], f32, tag="sume")
                    nc.vector.reduce_sum(
                        sume[:],
                        expT.rearrange("p j i -> p i j"),
                        axis=mybir.AxisListType.X,
                    )
                    return expT, sume

                def emit_tail(ic, expT, sume):
                    for isub in range(4):
                        it = ic * 4 + isub
                        # step 2: sum over the remaining 128 j-partitions
                        psd = pden.tile([P, 1], f32, tag="ps_d")
                        nc.tensor.matmul(
                            psd[:],
                            lhsT=sume[:, isub * P : (isub + 1) * P],
                            rhs=ones_t[:],
                            start=True,
                            stop=True,
                        )
                        rden = attn.tile([P, 1], f32, tag="rden", bufs=4)
                        nc.vector.reciprocal(rden[:], psd[:])
                        osb = attn.tile([P, E], f32, tag="osb", bufs=3)
                        for ch2 in range(2):
                            esl = slice(ch2 * 512, (ch2 + 1) * 512)
                            ps = pnum.tile([P, 512], f32, tag="ps_n")
                            for jt in range(NT):
                                nc.tensor.matmul(
                                    ps[:],
                                    lhsT=expT[:, jt, isub * P : (isub + 1) * P],
                                    rhs=vt[:, jt, esl],
                                    start=(jt == 0),
                                    stop=(jt == NT - 1),
                                )
                            nc.vector.tensor_scalar_mul(osb[:, esl], ps[:], rden[:])
                        nc.sync.dma_start(out_r[it], osb[:])

                prev = None
                for ic in range(NCH):
                    cur = emit_scores(ic)
                    if prev is not None:
                        emit_tail(ic - 1, *prev)
                    prev = cur
                emit_tail(NCH - 1, *prev)
    nc.compile()
    return nc


def get_nc():
    if "nc" not in _CACHE:
        _CACHE["nc"] = _build()
    return _CACHE["nc"]


def prepare_in_maps(x, W_qkv, b_qkv):
    bf = ml_dtypes.bfloat16
    x = np.asarray(x, dtype=np.float32)
    W = np.asarray(W_qkv, dtype=np.float32)
    b = np.asarray(b_qkv, dtype=np.float32)
    assert x.shape == (8, N, E) and W.shape == (F, E) and b.shape == (F,)
    xT = np.ascontiguousarray(np.transpose(x, (0, 2, 1))).astype(bf)  # [8, E, N]
    WT = np.ascontiguousarray(W.T).astype(bf)  # [E, F]
    bqk = np.ascontiguousarray(b[: 2 * E].reshape(16, P).T)  # [P, 16]
    bv = np.ascontiguousarray(np.broadcast_to(b[2 * E :], (P, E)))  # [P, E]
    return [{"xT": xT[i], "WT": WT, "b_qk": bqk, "b_v": bv} for i in range(8)]


def kernel(x, W_qkv, b_qkv):
    from concourse.bass_utils import run_bass_kernel_spmd

    nc = get_nc()
    in_maps = prepare_in_maps(x, W_qkv, b_qkv)
    res = run_bass_kernel_spmd(nc, in_maps, core_ids=list(range(8)))
    return np.stack([res.results[i]["out"] for i in range(8)], axis=0)
